# revision 1
# baseline (speedup 1.0000x reference)
"""BitCNN frontend on 8 trn2 cores — v2.

Data-parallel over batch (4 samples/core). Per layer: conv via TensorE with
A=gamma*r folded into runtime-scaled bf16 weights, B folded into the eviction
bias, and the cos-term fed as a second fp8 DoubleRow conv path whose weights
are exact ternary*0.5 (+ a small fp8 eps-correction for 0.5/a != 0.5).
Phase-2 per element: q-affine (DVE), mantissa mask (DVE/Pool), Sin (ACT) into
fp8 c. GroupNorm stats: Sum(y) rides eviction accum; Sum(y^2) from a stride-4
subsample of PSUM (exact mean, subsampled variance). L1 uses a host-built
im2col ([25-row windows packed 4x32 partitions]) so x is DMA'd ~once.
"""
import math
import os

import numpy as np
import ml_dtypes

import bass_rust as _br
import concourse.bass as bass
import concourse.tile as tile
from concourse import mybir
from concourse.bass_utils import run_bass_kernel_spmd

f32 = mybir.dt.float32
f32r = mybir.dt.float32r
bf16 = mybir.dt.bfloat16
fp8 = mybir.dt.float8e4
i32 = mybir.dt.int32
PS = bass.MemorySpace.PSUM
AF = mybir.ActivationFunctionType
ALU = mybir.AluOpType
BF = ml_dtypes.bfloat16
F8 = ml_dtypes.float8_e4m3
DR = mybir.MatmulPerfMode.DoubleRow

N_CORES = 8
BPC = 4
L_IN = 320000
EPS_GN = 1e-5

T1, T2, T3, T4 = 64001, 16001, 4001, 1001
U1, V2 = 16001, 8001
G1, G2, G3 = 1, 2, 4
W1B = G1 + U1 + 12         # y1/c1 buffer cols (slack for pad4 over-reads)
W2B = G2 + V2 + 13   # even total (fp8 c2 alignment)
W3B = G3 + T3 + 21   # even total (fp8 c3 alignment)
QW = 4096                   # xim quarter width
NL = [32 * T1, 64 * T2, 128 * T3, 256 * T4]

TILE = 512
CHUNK = int(os.environ.get("KN_CHUNK", "1536"))
PH2 = int(os.environ.get("KN_PH2", "1536"))

MASK_AND = 0x0007FFFF
MASK_OR = 0x4B000000
SIN_SCALE = 2.0 * math.pi / (2 ** 19)
NEGPI33 = -103.67255756846316  # -33*pi

# engine assignment (tunable): value in {"act", "dve"} / {"dve", "pool"}
EV_ENG = {int(k): v for k, v in zip("1234", os.environ.get("KN_EV", "aaaa"))}
EV_ENG = {k: {"a": "act", "d": "dve"}[v] for k, v in EV_ENG.items()}
MASK_ENG = {1: "dve", 2: "dve", 3: "dve", 4: "dve"}
COEF_ENG = {1: "dve", 2: "dve", 3: "dve", 4: "dve"}
INCLUDE_EPS = True
SQ_ENG = "dve"

_CACHE = {}
LAST_RESULTS = None


def _vp(pairs):
    return _br.VecI64Pair(pairs)


def split_multi_waits(nc):
    """Walrus accepts only ONE sem-wait per instruction; hoist extras onto
    same-engine NOPs placed just before the instruction."""
    eng_map = nc.engines
    for bass_bb in list(nc.bb_map.values()):
        bb = bass_bb.bb
        insts = list(bb.instructions)
        if not any(i.sync_info is not None and i.sync_info.on_wait
                   and len(i.sync_info.on_wait) > 1 for i in insts):
            continue
        newlist = []
        for inst in insts:
            si = inst.sync_info
            if si is not None and si.on_wait and len(si.on_wait) > 1:
                waits = list(si.on_wait)
                inst.sync_info = mybir.SyncInfo(
                    on_wait=waits[:1],
                    on_update=list(si.on_update) if si.on_update else [])
                eng = eng_map[inst.engine]
                for w in waits[1:]:
                    nop = eng.nop(nofuse=True)
                    cur = nc.cur_bb.bb
                    assert cur.instructions[-1] is nop.ins
                    cur.instructions = cur.instructions[:-1]
                    nop.ins.sync_info = mybir.SyncInfo(on_wait=[w], on_update=[])
                    newlist.append(nop.ins)
            newlist.append(inst)
        bb.instructions = newlist


# ---------------------------------------------------------------------------
# host-side preparation
# ---------------------------------------------------------------------------

def _ternary(w):
    s = np.float32(np.mean(np.abs(w), dtype=np.float32) + np.float32(1e-8))
    t = np.clip(np.round(w / s), -1.0, 1.0).astype(np.float32)
    return t, float(s)


def _chunks(width):
    """[(start, width)] eviction chunks of CHUNK cols."""
    out = []
    c = 0
    while c < width:
        out.append((c, min(CHUNK, width - c)))
        c += CHUNK
    return out


L1_CH = _chunks(U1)       # 11 chunks (last 641)
L2_CH = _chunks(V2)       # 6 (last 321, contains junk col 8000)
L3_CH = _chunks(T3)       # 3 (last 929)
L4_CH = [(0, T4)]         # per half


def _host_prep(inputs):
    w = [np.asarray(inputs[f"w{i}"], np.float32) for i in range(1, 5)]
    g = [np.asarray(inputs[f"g{i}"], np.float32) for i in range(1, 5)]
    b = [np.asarray(inputs[f"b{i}"], np.float32) for i in range(1, 5)]
    a = [np.asarray(inputs[f"a{i}"], np.float32) for i in range(1, 5)]
    ph = [np.asarray(inputs[f"ph{i}"], np.float32) for i in range(1, 5)]

    tern = [_ternary(x) for x in w]
    t = [x[0] for x in tern]
    s = [x[1] for x in tern]
    eps_eff = tuple(EPS_GN / (si * si) for si in s)

    TWO_PI = 2.0 * math.pi
    HALF_PI = math.pi / 2.0
    hb1 = [b[i] + 0.5 / a[i] for i in range(4)]          # beta + 1/(2a)
    eps1 = [(1.0 / a[i] - 1.0) for i in range(4)]        # 0.5/a = 0.5*(1+eps)

    # --- L1 weights: [25,128] replicated at 4 partition offsets
    w1 = np.zeros((128, 128), np.float32)
    for q in range(4):
        for j in range(4):
            for r in range(25):
                k = r - 5 * j
                if 0 <= k <= 9:
                    w1[32 * q + r, j * 32:j * 32 + 32] = t[0][:, 0, k]

    # --- y-path ternary bases (device-scaled by gamma*r per sample)
    base2 = np.zeros((128, 3 * 128), np.float32)   # p=(j,ci) -> (b, h,co)
    for p in range(128):
        j, ci = p // 32, p % 32
        for bb_ in range(3):
            for h in range(2):
                k = 4 * bb_ + j - 4 * h
                if 0 <= k <= 7:
                    base2[p, 128 * bb_ + 64 * h:128 * bb_ + 64 * h + 64] = \
                        t[1][:, ci, k]
    base3 = np.zeros((128, 4 * 128), np.float32)   # p=(h,ci) -> (b, co)
    for p in range(128):
        h, ci = p // 64, p % 64
        for bb_ in range(4):
            k = 2 * bb_ + h
            base3[p, 128 * bb_:128 * bb_ + 128] = t[2][:, ci, k]
    base4 = np.zeros((128, 16 * 128), np.float32)  # p=ci -> (h, b, co)
    for h in range(2):
        for bb_ in range(8):
            base4[:, (h * 8 + bb_) * 128:(h * 8 + bb_) * 128 + 128] = \
                t[3][128 * h:128 * h + 128, :, bb_].T

    # --- c-path DR fp8 k-tiles (main: ternary*0.5 exact; eps-corr quantized)
    def f8(x):
        return x.astype(F8)

    d2 = np.zeros((3, 128, 128), np.float32)
    for p in range(128):
        j, ci = p // 32, p % 32
        for bb_ in range(3):
            for h in range(2):
                k = 4 * bb_ + j - 4 * h
                if 0 <= k <= 7:
                    d2[bb_, p, 64 * h:64 * h + 64] = t[1][:, ci, k] * 0.5
    e2 = d2 * eps1[0][np.arange(128) % 32][None, :, None]
    # mm order: [D0,D1], [D2,E2], [E0,E1]
    cdr2 = np.concatenate([d2[0], d2[1], d2[2], e2[2], e2[0], e2[1]], axis=1)

    d3 = np.zeros((4, 128, 128), np.float32)
    for p in range(128):
        h, ci = p // 64, p % 64
        for bb_ in range(4):
            d3[bb_, p, :] = t[2][:, ci, 2 * bb_ + h] * 0.5
    e3 = d3 * eps1[1][np.arange(128) % 64][None, :, None]
    cdr3 = np.concatenate([d3[0], d3[1], d3[2], d3[3],
                           e3[0], e3[1], e3[2], e3[3]], axis=1)

    d4 = np.zeros((2, 8, 128, 128), np.float32)
    for h in range(2):
        for bb_ in range(8):
            d4[h, bb_] = t[3][128 * h:128 * h + 128, :, bb_].T * 0.5
    e4 = d4 * eps1[2][None, None, :, None]
    blocks = []
    for h in range(2):
        for bb_ in range(8):
            blocks.append(d4[h, bb_])
        for bb_ in range(8):
            blocks.append(e4[h, bb_])
    cdr4 = np.concatenate(blocks, axis=1)

    # --- bias-matmul lhsTs: M[ci, out] = (sum_k t)[out,ci] * gamma[ci]
    def msum(tw, ks):
        return tw[:, :, ks].sum(axis=2)  # [co, ci]

    m2 = np.zeros((32, 3 * 128), np.float32)
    for (col, ks, hmask) in ((0, range(8), (0, 1)), (1, range(4), (0,)),
                             (2, range(5, 8), (0,))):
        sm = msum(t[1], list(ks)) * g[0][None, :]      # [64co, 32ci]
        for h in hmask:
            m2[:, col * 128 + 64 * h: col * 128 + 64 * h + 64] = sm.T
    m3 = np.zeros((64, 3 * 128), np.float32)
    for (col, ks) in ((0, range(8)), (1, range(4)), (2, range(5, 8))):
        sm = msum(t[2], list(ks)) * g[1][None, :]      # [128co, 64ci]
        m3[:, col * 128: col * 128 + 128] = sm.T
    m4 = np.zeros((128, 6 * 128), np.float32)
    for h in range(2):
        for (col, ks) in ((0, range(8)), (1, range(4)), (2, range(5, 8))):
            sm = msum(t[3][128 * h:128 * h + 128], list(ks)) * g[2][None, :]
            m4[:, (h * 3 + col) * 128: (h * 3 + col) * 128 + 128] = sm.T

    # --- host vec bundle [128, NV] f32
    cols = []

    def addcol(v):
        cols.append(np.asarray(v, np.float32).reshape(128))
        return len(cols) - 1

    ix = {}
    perm1 = np.arange(128) % 32
    perm2 = np.arange(128) % 64
    perm3 = np.arange(128)
    for li, pm in ((0, perm1), (1, perm2), (2, perm3)):
        ix[f"hj{li+1}"] = addcol(((2 * a[li] * g[li]) / TWO_PI)[pm])
        ix[f"p0{li+1}"] = addcol(
            ((2 * a[li] * b[li] + 2 * ph[li] + HALF_PI) / TWO_PI + 24.0)[pm])
    for h in range(2):
        sl = slice(128 * h, 128 * h + 128)
        ix[f"hj4{h}"] = addcol(((2 * a[3] * g[3]) / TWO_PI)[sl])
        ix[f"p04{h}"] = addcol(
            ((2 * a[3] * b[3] + 2 * ph[3] + HALF_PI) / TWO_PI + 24.0)[sl])
        ix[f"hb14{h}"] = addcol(hb1[3][sl])
        ix[f"hg4{h}"] = addcol(g[3][sl])
        ix[f"hC4{h}"] = addcol((0.5 / a[3])[sl])
    ix["fac2"] = addcol(g[0][perm1])
    ix["fac3"] = addcol(g[1][perm2])
    ix["fac4"] = addcol(g[2][perm3])
    # W1 vectors: sum_k t_next[out,ci]*hb1[ci], per next-layer out layout
    w1n2 = (msum(t[1], list(range(8))) @ hb1[0])       # [64]
    w1e02 = (msum(t[1], list(range(4))) @ hb1[0])
    w1eL2 = (msum(t[1], list(range(5, 8))) @ hb1[0])
    ix["W1n2"] = addcol(np.concatenate([w1n2, w1n2]))
    ix["W1e02"] = addcol(np.concatenate([w1e02, np.zeros(64)]))
    ix["W1eL2"] = addcol(np.concatenate([w1eL2, np.zeros(64)]))
    ix["W1n3"] = addcol(msum(t[2], list(range(8))) @ hb1[1])
    ix["W1e03"] = addcol(msum(t[2], list(range(4))) @ hb1[1])
    ix["W1eL3"] = addcol(msum(t[2], list(range(5, 8))) @ hb1[1])
    for h in range(2):
        th = t[3][128 * h:128 * h + 128]
        ix[f"W1n4{h}"] = addcol(msum(th, list(range(8))) @ hb1[2])
        ix[f"W1e04{h}"] = addcol(msum(th, list(range(4))) @ hb1[2])
        ix[f"W1eL4{h}"] = addcol(msum(th, list(range(5, 8))) @ hb1[2])
    # per-partition real counts and subsample scales
    np1 = np.where(perm1 == perm1, 0, 0)  # placeholder
    cnt1 = np.full(128, T1 // 4, np.float64)
    cnt1[np.arange(128) // 32 == 0] = T1 - 3 * (T1 // 4)  # j=0: 16001
    cnt1 = np.where(np.arange(128) // 32 == 0, 16001, 16000)
    sub1 = np.where(np.arange(128) // 32 == 0, 4001, 4000)
    ix["r441"] = addcol(cnt1 / sub1)
    cnt2 = np.where(np.arange(128) < 64, 8001, 8000)
    ix["Np2"] = addcol(cnt2)
    ix["r442"] = addcol(cnt2 / 2000.0)
    ix["Np3"] = addcol(np.full(128, 4001.0))
    ix["r443"] = addcol(np.full(128, 4001.0 / 1001.0))
    ix["Np4"] = addcol(np.full(128, 2002.0))
    ix["r444"] = addcol(np.full(128, 2002.0 / 502.0))
    hv = np.stack(cols, axis=1)

    host = {
        "w1t": w1.astype(BF),
        "base2": base2.astype(BF),
        "base3": base3.astype(BF),
        "base4": base4.astype(BF),
        "cdr2": f8(cdr2), "cdr3": f8(cdr3), "cdr4": f8(cdr4),
        "m2": np.ascontiguousarray(m2), "m3": np.ascontiguousarray(m3),
        "m4": np.ascontiguousarray(m4),
        "hv": np.ascontiguousarray(hv),
        "eye": np.eye(128).astype(BF),
    }
    return host, eps_eff, ix


def _build_im2col(xs):
    """xs [BPC, L_IN] f32 -> [BPC, 128, QW] bf16 quarter-packed im2col."""
    out = np.zeros((BPC, 128, QW), np.float32)
    idx_u = np.arange(QW)
    for q in range(4):
        u = QW * q + idx_u
        valid_u = u < U1
        for r in range(25):
            pos = 20 * u - 5 + r
            ok = valid_u & (pos >= 0) & (pos < L_IN)
            out[:, 32 * q + r, ok] = xs[:, pos[ok]]
    return np.ascontiguousarray(out.astype(BF))


# ---------------------------------------------------------------------------
# device program
# ---------------------------------------------------------------------------

def _pad4(n):
    return (n + 3) // 4 * 4


def _build_program(eps_eff, ix, nv):
    nc = bass.Bass()
    xim_d = nc.dram_tensor("xim", (BPC, 128, QW), bf16, kind="ExternalInput")
    w1_d = nc.dram_tensor("w1t", (128, 128), bf16, kind="ExternalInput")
    b2_d = nc.dram_tensor("base2", (128, 384), bf16, kind="ExternalInput")
    b3_d = nc.dram_tensor("base3", (128, 512), bf16, kind="ExternalInput")
    b4_d = nc.dram_tensor("base4", (128, 2048), bf16, kind="ExternalInput")
    c2_d = nc.dram_tensor("cdr2", (128, 768), fp8, kind="ExternalInput")
    c3_d = nc.dram_tensor("cdr3", (128, 1024), fp8, kind="ExternalInput")
    c4_d = nc.dram_tensor("cdr4", (128, 4096), fp8, kind="ExternalInput")
    m2_d = nc.dram_tensor("m2", (32, 384), f32, kind="ExternalInput")
    m3_d = nc.dram_tensor("m3", (64, 384), f32, kind="ExternalInput")
    m4_d = nc.dram_tensor("m4", (128, 768), f32, kind="ExternalInput")
    hv_d = nc.dram_tensor("hv", (128, nv), f32, kind="ExternalInput")
    eye_d = nc.dram_tensor("eye", (128, 128), bf16, kind="ExternalInput")
    out_d = nc.dram_tensor("out", (BPC, T4, 256), bf16, kind="ExternalOutput")
    DBG = os.environ.get("KN_DEBUG", "0") == "1"
    if DBG:
        dbg = {}
        for nm, wd, dt_ in (("y1", W1B, bf16), ("c1", W1B, fp8),
                            ("y2", W2B, bf16), ("c2", W2B, fp8),
                            ("y3", W3B, bf16), ("c3", W3B, fp8),
                            ("y4", 2002, bf16), ("c4", 2002, bf16)):
            dbg[nm] = nc.dram_tensor(f"dbg_{nm}", (128, wd), dt_,
                                     kind="ExternalOutput")
        dbg["bv2"] = nc.dram_tensor("dbg_bv2", (128, 4), f32,
                                    kind="ExternalOutput")
        dbg["co2"] = nc.dram_tensor("dbg_co2", (128, 8), f32,
                                    kind="ExternalOutput")

    with tile.TileContext(nc) as tc:
        with (
            tc.tile_pool(name="big", bufs=1) as big,
            tc.tile_pool(name="wp", bufs=1) as wp,
            tc.tile_pool(name="xp", bufs=1) as xp,
            tc.tile_pool(name="qp", bufs=int(os.environ.get("KN_QB", "3"))) as qp,
            tc.tile_pool(name="sqp", bufs=int(os.environ.get("KN_SB", "2"))) as sqp,
            tc.tile_pool(name="wsc", bufs=int(os.environ.get("KN_WB", "2"))) as wsc,
            tc.tile_pool(name="coefp", bufs=int(os.environ.get("KN_CB", "2"))) as coefp,
            tc.tile_pool(name="slotp", bufs=int(os.environ.get("KN_SLB", "2"))) as slotp,
            tc.tile_pool(name="outp", bufs=1) as outp,
            tc.tile_pool(name="ps", bufs=2, space=PS) as psp,
            tc.tile_pool(name="psb", bufs=1, space=PS) as psbp,
            tc.tile_pool(name="pst", bufs=1, space=PS) as pstp,
        ):
            y1 = big.tile([128, W1B], bf16)
            c1 = big.tile([128, W1B], fp8)
            y2 = big.tile([128, 2 * W2B], bf16)
            c2 = big.tile([128, 2 * W2B], fp8)
            y3 = big.tile([128, 2 * W3B], bf16)
            c3 = big.tile([128, 2 * W3B], fp8)
            y4 = big.tile([128, 2 * 2002], bf16)
            c4 = big.tile([128, 2 * 2002], bf16)
            ones = big.tile([128, 128], f32)
            negpi = big.tile([128, 1], f32)

            w1t = wp.tile([128, 128], bf16)
            base2 = wp.tile([128, 384], bf16)
            base3 = wp.tile([128, 512], bf16)
            base4 = wp.tile([128, 2048], bf16)
            cdr2 = wp.tile([128, 768], fp8)
            cdr3 = wp.tile([128, 1024], fp8)
            cdr4 = wp.tile([128, 4096], fp8)
            m2t = wp.tile([128, 384], f32)
            m3t = wp.tile([128, 384], f32)
            m4t = wp.tile([128, 768], f32)
            hv = wp.tile([128, nv], f32)
            eye = wp.tile([128, 128], bf16)

            for dst, src in ((w1t, w1_d), (base2, b2_d), (base3, b3_d),
                             (base4, b4_d), (cdr2, c2_d), (cdr3, c3_d),
                             (cdr4, c4_d), (hv, hv_d), (eye, eye_d)):
                nc.sync.dma_start(dst[:], src[:])
            nc.sync.dma_start(m2t[0:32, :], m2_d[:])
            nc.sync.dma_start(m3t[0:64, :], m3_d[:])
            nc.sync.dma_start(m4t[0:128, 0:768], m4_d[:])
            nc.vector.memset(ones[:], 1.0)
            nc.vector.memset(negpi[:], NEGPI33)
            # zero guards (evictions never touch them)
            nc.gpsimd.memset(y1[:, 0:G1], 0.0)
            nc.gpsimd.memset(y1[:, G1 + U1:W1B], 0.0)
            nc.gpsimd.memset(c1[:, 0:G1], 0.0)
            nc.gpsimd.memset(c1[:, G1 + U1:W1B], 0.0)
            for sl in range(2):
                o2 = sl * W2B
                o3 = sl * W3B
                nc.gpsimd.memset(y2[:, o2:o2 + G2], 0.0)
                nc.gpsimd.memset(y2[:, o2 + G2 + V2:o2 + W2B], 0.0)
                nc.gpsimd.memset(c2[:, o2:o2 + G2], 0.0)
                nc.gpsimd.memset(c2[:, o2 + G2 + V2:o2 + W2B], 0.0)
                nc.gpsimd.memset(y3[:, o3:o3 + G3], 0.0)
                nc.gpsimd.memset(y3[:, o3 + G3 + T3:o3 + W3B], 0.0)
                nc.gpsimd.memset(c3[:, o3:o3 + G3], 0.0)
                nc.gpsimd.memset(c3[:, o3 + G3 + T3:o3 + W3B], 0.0)

            cdr2v = cdr2[:].rearrange("p (k m) -> p k m", m=128)
            cdr3v = cdr3[:].rearrange("p (k m) -> p k m", m=128)
            cdr4v = cdr4[:].rearrange("p (k m) -> p k m", m=128)

            def hvc(name):
                return hv[:, ix[name]:ix[name] + 1]

            ENG = {"dve": nc.vector, "pool": nc.gpsimd}

            def evict(li, ps_ap, dst_ap, slot_ap, bias_ap):
                if EV_ENG[li] == "act":
                    nc.scalar.activation(dst_ap, ps_ap, AF.Identity,
                                         bias=bias_ap if bias_ap is not None else 0.0,
                                         scale=1.0, accum_out=slot_ap)
                else:
                    nc.vector.tensor_scalar(
                        dst_ap, ps_ap,
                        bias_ap if bias_ap is not None else 0.0, None,
                        op0=ALU.add, op1=ALU.add, accum_out=slot_ap)

            def sqpass(y_ap_s4, w4, slot_ap):
                st = sqp.tile([128, CHUNK // 4], bf16, tag="sq")
                ENG[SQ_ENG].scalar_tensor_tensor(st[:, 0:w4], y_ap_s4, 1.0,
                                                 y_ap_s4, op0=ALU.mult,
                                                 op1=ALU.mult, accum_out=slot_ap)

            def ph2(li, s, ybuf, yoff, width, cbuf, coff, scl_ap, bis_ap,
                    cdt_width=None):
                for ch0 in range(0, width, PH2):
                    w = min(PH2, width - ch0)
                    qt = qp.tile([128, PH2], f32, tag="q")
                    nc.vector.tensor_scalar(qt[:, 0:w],
                                            ybuf[:, yoff + ch0:yoff + ch0 + w],
                                            scl_ap, bis_ap,
                                            op0=ALU.mult, op1=ALU.add)
                    qb = qt[:, 0:w].bitcast(i32)
                    ENG[MASK_ENG[li]].tensor_scalar(
                        qb, qb, MASK_AND, MASK_OR,
                        op0=ALU.bitwise_and, op1=ALU.bitwise_or)
                    nc.scalar.activation(cbuf[:, coff + ch0:coff + ch0 + w],
                                         qt[:, 0:w], AF.Sin,
                                         bias=negpi[:, 0:1], scale=SIN_SCALE)

            # --- conv emitters (chunk closures) --------------------------
            def l1_conv_chunks(ximt, sy, sq_):
                out = []
                for ci_, (c0, cw) in enumerate(L1_CH):
                    def fn(ci_=ci_, c0=c0, cw=cw):
                        acc = psp.tile([128, CHUNK], f32, tag="ps")
                        u0 = c0
                        off = 0
                        while off < cw:
                            wdt = min(TILE, cw - off)
                            wpd = _pad4(wdt)
                            q = u0 // QW
                            lo = u0 - QW * q
                            nc.tensor.matmul(acc[:, off:off + wpd],
                                             w1t[32 * q:32 * q + 25, :],
                                             ximt[32 * q:32 * q + 25, lo:lo + wpd],
                                             start=True, stop=True,
                                             tile_position=(32 * q, 0))
                            off += wdt
                            u0 += wdt
                        evict(1, acc[:, 0:cw], y1[:, G1 + c0:G1 + c0 + cw],
                              sy[:, ci_:ci_ + 1], None)
                        sqpass(y1[:, G1 + c0:G1 + c0 + cw:4],
                               (cw + 3) // 4, sq_[:, ci_:ci_ + 1])
                    out.append((0, fn))
                return out

            def l2_conv_chunks(sl, l2y, bias_ap, e0_ap, eL_ap, sy, sq_):
                o2 = sl * W2B
                out = []
                nch = len(L2_CH)
                for ci_, (c0, cw) in enumerate(L2_CH):
                    last = ci_ == nch - 1
                    def fn(ci_=ci_, c0=c0, cw=cw, last=last):
                        acc = psp.tile([128, CHUNK], f32, tag="ps")
                        off = 0
                        while off < cw:
                            wdt = min(TILE, cw - off)
                            wpd = _pad4(wdt)
                            v0 = c0 + off
                            base = G1 + 2 * v0 - 1
                            mms = []
                            for b in range(3):
                                rhs = y1[:, base + b: base + b + 2 * wpd:2]
                                mms.append((l2y[:, 128 * b:128 * b + 128], rhs, None))
                            cb = c1[:, base:base + 2 * wpd]
                            cbv = cb.rearrange("p (n two) -> p two n", two=2)
                            mms.append((cdr2v[:, 0:2, :], cbv, DR))
                            cb2 = c1[:, base + 2:base + 2 + 2 * wpd:2]
                            cb2 = cb2.unsqueeze(1).broadcast_to([128, 2, wpd])
                            mms.append((cdr2v[:, 2:4, :], cb2, DR))
                            if INCLUDE_EPS:
                                mms.append((cdr2v[:, 4:6, :], cbv, DR))
                            for mi, (lw, rh, pm) in enumerate(mms):
                                kw = dict(start=(mi == 0), stop=(mi == len(mms) - 1))
                                if pm is not None:
                                    kw["perf_mode"] = pm
                                nc.tensor.matmul(acc[:, off:off + wpd], lw, rh, **kw)
                            off += wdt
                        if not last:
                            evict(2, acc[:, 0:cw], y2[:, o2 + G2 + c0:o2 + G2 + c0 + cw],
                                  sy[:, ci_:ci_ + 1], bias_ap)
                            sqpass(y2[:, o2 + G2 + c0:o2 + G2 + c0 + cw:4],
                                   (cw + 3) // 4, sq_[:, ci_:ci_ + 1])
                        else:
                            evict(2, acc[:, 0:cw - 1],
                                  y2[:, o2 + G2 + c0:o2 + G2 + c0 + cw - 1],
                                  sy[:, ci_:ci_ + 1], bias_ap)
                            for p0 in (64, 96):
                                nc.vector.memset(sy[p0:p0 + 32, ci_ + 1:ci_ + 2], 0.0)
                            evict(2, acc[0:64, cw - 1:cw],
                                  y2[0:64, o2 + G2 + 8000:o2 + G2 + 8001],
                                  sy[0:64, ci_ + 1:ci_ + 2], bias_ap[0:64, 0:1])
                            for p0 in (64, 96):
                                nc.gpsimd.memset(y2[p0:p0 + 32, o2 + G2 + 8000:o2 + G2 + 8001], 0.0)
                            sqpass(y2[:, o2 + G2 + c0:o2 + G2 + c0 + cw - 1:4],
                                   (cw - 1 + 3) // 4, sq_[:, ci_:ci_ + 1])
                        if ci_ == 0:
                            nc.vector.tensor_tensor(y2[:, o2 + G2:o2 + G2 + 1],
                                                    y2[:, o2 + G2:o2 + G2 + 1],
                                                    e0_ap, op=ALU.subtract)
                        if last:
                            nc.vector.tensor_tensor(
                                y2[0:64, o2 + G2 + 8000:o2 + G2 + 8001],
                                y2[0:64, o2 + G2 + 8000:o2 + G2 + 8001],
                                eL_ap[0:64, 0:1], op=ALU.subtract)
                    out.append((2 * (c0 + cw) + 10, fn))
                return out

            def l3_conv_chunks(sl, l3y, bias_ap, e0_ap, eL_ap, sy, sq_):
                o2 = sl * W2B
                o3 = sl * W3B
                out = []
                for ci_, (c0, cw) in enumerate(L3_CH):
                    def fn(ci_=ci_, c0=c0, cw=cw):
                        acc = psp.tile([128, CHUNK], f32, tag="ps")
                        off = 0
                        while off < cw:
                            wdt = min(TILE, cw - off)
                            wpd = _pad4(wdt)
                            t0 = c0 + off
                            base = o2 + G2 + 2 * t0 - 2
                            mms = []
                            for b in range(4):
                                rhs = y2[:, base + b: base + b + 2 * wpd:2]
                                mms.append((l3y[:, 128 * b:128 * b + 128], rhs, None))
                            cb = c2[:, base:base + 2 * wpd]
                            cbv = cb.rearrange("p (n two) -> p two n", two=2)
                            cb2 = c2[:, base + 2:base + 2 + 2 * wpd]
                            cb2v = cb2.rearrange("p (n two) -> p two n", two=2)
                            mms.append((cdr3v[:, 0:2, :], cbv, DR))
                            mms.append((cdr3v[:, 2:4, :], cb2v, DR))
                            if INCLUDE_EPS:
                                mms.append((cdr3v[:, 4:6, :], cbv, DR))
                                mms.append((cdr3v[:, 6:8, :], cb2v, DR))
                            for mi, (lw, rh, pm) in enumerate(mms):
                                kw = dict(start=(mi == 0), stop=(mi == len(mms) - 1))
                                if pm is not None:
                                    kw["perf_mode"] = pm
                                nc.tensor.matmul(acc[:, off:off + wpd], lw, rh, **kw)
                            off += wdt
                        evict(3, acc[:, 0:cw], y3[:, o3 + G3 + c0:o3 + G3 + c0 + cw],
                              sy[:, ci_:ci_ + 1], bias_ap)
                        sqpass(y3[:, o3 + G3 + c0:o3 + G3 + c0 + cw:4],
                               (cw + 3) // 4, sq_[:, ci_:ci_ + 1])
                        if ci_ == 0:
                            nc.vector.tensor_tensor(y3[:, o3 + G3:o3 + G3 + 1],
                                                    y3[:, o3 + G3:o3 + G3 + 1],
                                                    e0_ap, op=ALU.subtract)
                        if ci_ == len(L3_CH) - 1:
                            nc.vector.tensor_tensor(
                                y3[:, o3 + G3 + 4000:o3 + G3 + 4001],
                                y3[:, o3 + G3 + 4000:o3 + G3 + 4001],
                                eL_ap, op=ALU.subtract)
                    out.append((2 * (c0 + cw) + 10, fn))
                return out

            def l4_conv_chunks(sl, l4y, bias_aps, e0_aps, eL_aps, sy, sq_):
                o3 = sl * W3B
                o4 = sl * 2002
                out = []
                for h in range(2):
                    def fn(h=h):
                        acc = psp.tile([128, CHUNK], f32, tag="ps")
                        off = 0
                        while off < T4:
                            wdt = min(TILE, T4 - off)
                            wpd = _pad4(wdt)
                            t0 = off
                            base = o3 + G3 + 4 * t0 - 4
                            mms = []
                            for b in range(8):
                                rhs = y3[:, base + b: base + b + 4 * wpd:4]
                                mms.append((l4y[:, (8 * h + b) * 128:(8 * h + b) * 128 + 128],
                                            rhs, None))
                            for j in range(4):
                                cb = c3[:, base + 2 * j:base + 2 * j + 4 * wpd]
                                cbv = cb.rearrange("p (n four) -> p four n", four=4)
                                mms.append((cdr4v[:, 16 * h + 2 * j:16 * h + 2 * j + 2, :],
                                            cbv[:, 0:2, :], DR))
                            if INCLUDE_EPS:
                                for j in range(4):
                                    cb = c3[:, base + 2 * j:base + 2 * j + 4 * wpd]
                                    cbv = cb.rearrange("p (n four) -> p four n", four=4)
                                    mms.append((cdr4v[:, 16 * h + 8 + 2 * j:16 * h + 8 + 2 * j + 2, :],
                                                cbv[:, 0:2, :], DR))
                            for mi, (lw, rh, pm) in enumerate(mms):
                                kw = dict(start=(mi == 0), stop=(mi == len(mms) - 1))
                                if pm is not None:
                                    kw["perf_mode"] = pm
                                nc.tensor.matmul(acc[:, off:off + wpd], lw, rh, **kw)
                            off += wdt
                        evict(4, acc[:, 0:T4], y4[:, o4 + 1001 * h:o4 + 1001 * h + T4],
                              sy[:, h:h + 1], bias_aps[h])
                        sqpass(y4[:, o4 + 1001 * h:o4 + 1001 * h + T4:4],
                               (T4 + 3) // 4, sq_[:, h:h + 1])
                        nc.vector.tensor_tensor(y4[:, o4 + 1001 * h:o4 + 1001 * h + 1],
                                                y4[:, o4 + 1001 * h:o4 + 1001 * h + 1],
                                                e0_aps[h], op=ALU.subtract)
                        nc.vector.tensor_tensor(
                            y4[:, o4 + 1001 * h + 1000:o4 + 1001 * h + 1001],
                            y4[:, o4 + 1001 * h + 1000:o4 + 1001 * h + 1001],
                            eL_aps[h], op=ALU.subtract)
                    out.append((4 * T4 + 10, fn))
                return out

            def ph2_chunks(li, ybuf, yoff, width, cbuf, coff, scl_ap, bis_ap,
                           post=None):
                out = []
                nchk = (width + PH2 - 1) // PH2
                for i in range(nchk):
                    ch0 = i * PH2
                    w = min(PH2, width - ch0)
                    def fn(ch0=ch0, w=w, lastc=(i == nchk - 1)):
                        qt = qp.tile([128, PH2], f32, tag="q")
                        nc.vector.tensor_scalar(qt[:, 0:w],
                                                ybuf[:, yoff + ch0:yoff + ch0 + w],
                                                scl_ap, bis_ap,
                                                op0=ALU.mult, op1=ALU.add)
                        qb = qt[:, 0:w].bitcast(i32)
                        ENG[MASK_ENG[li]].tensor_scalar(
                            qb, qb, MASK_AND, MASK_OR,
                            op0=ALU.bitwise_and, op1=ALU.bitwise_or)
                        nc.scalar.activation(cbuf[:, coff + ch0:coff + ch0 + w],
                                             qt[:, 0:w], AF.Sin,
                                             bias=negpi[:, 0:1], scale=SIN_SCALE)
                        if lastc and post is not None:
                            post()
                    out.append((ch0 + w, fn))
                return out

            def weave(p_chunks, c_chunks):
                ci_ = 0
                for covered, fn in p_chunks:
                    fn()
                    while ci_ < len(c_chunks) and c_chunks[ci_][0] <= covered:
                        c_chunks[ci_][1]()
                        ci_ += 1
                for _, fn in c_chunks[ci_:]:
                    fn()

            # --- coefficient blocks --------------------------------------
            # state dicts keyed by sample
            ST = {"bvec2": {}, "e02": {}, "eL2": {}, "nb2": {},
                  "bvec3": {}, "e03": {}, "eL3": {}, "nb3": {},
                  "bvec4": {}, "e04": {}, "eL4": {}, "nb4": {},
                  "scl": {}, "bis": {}, "A4": {}, "B4z": {},
                  "l2y": {}, "l3y": {}, "l4y": {}}

            def emit_coefs_fast(li, ss):
                e = nc.vector
                for k, (s_, syt, sqt, nsl) in enumerate(ss):
                    if li < 4:
                        sclT = coefp.tile([128, 1], f32, tag=f"scl{li}{k}")
                        bisT = coefp.tile([128, 1], f32, tag=f"bis{li}{k}")
                        e.memset(sclT[:], 0.01)
                        e.memset(bisT[:], 24.0)
                        ST["scl"][(li, s_)] = sclT
                        ST["bis"][(li, s_)] = bisT
                        nli = li + 1
                        lcols = {2: 384, 3: 512, 4: 2048}[nli]
                        lw = wsc.tile([128, lcols], bf16, tag=f"l{nli}y")
                        e.memset(lw[:], 0.01)
                        ST[f"l{nli}y"][s_] = lw
                        if nli < 4:
                            bvT = coefp.tile([128, 1], f32, tag=f"bv{nli}{k}")
                            e.memset(bvT[:], 0.0)
                            ST[f"bvec{nli}"][s_] = bvT[:]
                            ST[f"e0{nli}"][s_] = bvT[:]
                            ST[f"eL{nli}"][s_] = bvT[:]
                        else:
                            bvT = coefp.tile([128, 1], f32, tag=f"bv4x{k}")
                            e.memset(bvT[:], 0.0)
                            ST["bvec4"][s_] = [bvT[:], bvT[:]]
                            ST["e04"][s_] = [bvT[:], bvT[:]]
                            ST["eL4"][s_] = [bvT[:], bvT[:]]
                    else:
                        ST["scl"][(4, s_)] = {}
                        ST["bis"][(4, s_)] = {}
                        ST["A4"][s_] = {}
                        ST["B4z"][s_] = {}
                        for h in range(2):
                            sclT = coefp.tile([128, 1], f32, tag=f"scl4{h}{k}")
                            e.memset(sclT[:], 0.01)
                            ST["scl"][(4, s_)][h] = sclT
                            ST["bis"][(4, s_)][h] = sclT
                            ST["A4"][s_][h] = sclT
                            ST["B4z"][s_][h] = sclT

            FASTCOEF = os.environ.get("KN_FASTCOEF", "0") == "1"

            def emit_coefs(li, ss):
                """ss: list of (s, sy_tile, sq_tile, (nsl_y, nsl_q))."""
                if FASTCOEF:
                    emit_coefs_fast(li, ss)
                    return
                e = ENG[COEF_ENG[li]]
                w = len(ss)
                st = coefp.tile([128, 4], f32, tag=f"st{li}")
                for k, (s, syt, sqt, nsl) in enumerate(ss):
                    nc.vector.tensor_reduce(st[:, k:k + 1], syt[:, 0:nsl[0]],
                                            axis=mybir.AxisListType.X, op=ALU.add)
                    nc.vector.tensor_reduce(st[:, w + k:w + k + 1], sqt[:, 0:nsl[1]],
                                            axis=mybir.AxisListType.X, op=ALU.add)
                e.tensor_scalar(st[:, w:2 * w], st[:, w:2 * w],
                                hvc(f"r44{li}"), None, op0=ALU.mult)
                accp = psbp.tile([128, 12], f32, tag="psb")
                nc.tensor.matmul(accp[:, 0:2 * w], ones[:], st[:, 0:2 * w],
                                 start=True, stop=True)
                ms = coefp.tile([128, 4], f32, tag=f"ms{li}")
                nc.vector.tensor_scalar(ms[:, 0:w], accp[:, 0:w], 1.0 / NL[li - 1],
                                        None, op0=ALU.mult)
                nc.vector.tensor_scalar(ms[:, w:2 * w], accp[:, w:2 * w],
                                        1.0 / NL[li - 1], eps_eff[li - 1],
                                        op0=ALU.mult, op1=ALU.add)
                ve = coefp.tile([128, 2], f32, tag=f"ve{li}")
                e.tensor_tensor(ve[:, 0:w], ms[:, 0:w], ms[:, 0:w], op=ALU.mult)
                e.tensor_tensor(ve[:, 0:w], ms[:, w:2 * w], ve[:, 0:w],
                                op=ALU.subtract)
                rt = coefp.tile([128, 2], f32, tag=f"rt{li}")
                if os.environ.get("KN_RSQ", "act") == "newton":
                    seed = coefp.tile([128, 2], i32, tag=f"sd{li}")
                    nc.vector.tensor_scalar(seed[:, 0:w], ve[:, 0:w].bitcast(i32),
                                            1, None, op0=ALU.arith_shift_right)
                    nc.vector.tensor_scalar(seed[:, 0:w], seed[:, 0:w], -1,
                                            0x5F3759DF, op0=ALU.mult, op1=ALU.add)
                    nc.vector.tensor_copy(rt[:, 0:w], seed[:, 0:w].bitcast(f32))
                    for _it in range(3):
                        rsq = coefp.tile([128, 2], f32, tag=f"rs{li}")
                        nc.vector.scalar_tensor_tensor(rsq[:, 0:w], rt[:, 0:w], 1.0,
                                                       rt[:, 0:w], op0=ALU.mult,
                                                       op1=ALU.mult)
                        nc.vector.tensor_tensor(rsq[:, 0:w], rsq[:, 0:w],
                                                ve[:, 0:w], op=ALU.mult)
                        nc.vector.tensor_scalar(rsq[:, 0:w], rsq[:, 0:w], -0.5, 1.5,
                                                op0=ALU.mult, op1=ALU.add)
                        nc.vector.tensor_tensor(rt[:, 0:w], rt[:, 0:w], rsq[:, 0:w],
                                                op=ALU.mult)
                else:
                    rv = coefp.tile([128, 2], f32, tag=f"rv{li}")
                    nc.vector.reciprocal(rv[:, 0:w], ve[:, 0:w])
                    nc.scalar.activation(rt[:, 0:w], rv[:, 0:w], AF.Sqrt,
                                         bias=0.0, scale=1.0)
                for k, (s, _, _, _) in enumerate(ss):
                    r_ap = rt[:, k:k + 1]
                    mu_ap = ms[:, k:k + 1]
                    # mpb = mu + current-layer evict bias (per partition)
                    mpb_ap = mu_ap
                    mpb4 = [mu_ap, mu_ap] if li == 4 else None
                    if li < 4:
                        sclT = coefp.tile([128, 1], f32, tag=f"scl{li}{k}")
                        e.tensor_tensor(sclT[:], hvc(f"hj{li}"), r_ap, op=ALU.mult)
                        tsc = coefp.tile([128, 1], f32, tag=f"tsc{li}{k}")
                        e.tensor_tensor(tsc[:], sclT[:], mpb_ap, op=ALU.mult)
                        bisT = coefp.tile([128, 1], f32, tag=f"bis{li}{k}")
                        nc.vector.scalar_tensor_tensor(bisT[:], tsc[:], -1.0,
                                                       hvc(f"p0{li}"),
                                                       op0=ALU.mult, op1=ALU.add)
                        ST["scl"][(li, s)] = sclT
                        ST["bis"][(li, s)] = bisT
                    else:
                        ST["scl"][(4, s)] = {}
                        ST["bis"][(4, s)] = {}
                        ST["A4"][s] = {}
                        ST["B4z"][s] = {}
                        for h in range(2):
                            sclT = coefp.tile([128, 1], f32, tag=f"scl4{h}{k}")
                            e.tensor_tensor(sclT[:], hvc(f"hj4{h}"), r_ap,
                                            op=ALU.mult)
                            tsc = coefp.tile([128, 1], f32, tag=f"tsc4{h}{k}")
                            e.tensor_tensor(tsc[:], sclT[:], mpb4[h],
                                            op=ALU.mult)
                            bisT = coefp.tile([128, 1], f32, tag=f"bis4{h}{k}")
                            nc.vector.scalar_tensor_tensor(bisT[:], tsc[:], -1.0,
                                                           hvc(f"p04{h}"),
                                                           op0=ALU.mult, op1=ALU.add)
                            A4T = coefp.tile([128, 1], f32, tag=f"A4{h}{k}")
                            e.tensor_tensor(A4T[:], hvc(f"hg4{h}"), r_ap,
                                            op=ALU.mult)
                            t4_ = coefp.tile([128, 1], f32, tag=f"t4{h}{k}")
                            e.tensor_tensor(t4_[:], A4T[:], mpb4[h],
                                            op=ALU.mult)
                            B4T = coefp.tile([128, 1], f32, tag=f"B4{h}{k}")
                            nc.vector.scalar_tensor_tensor(B4T[:], t4_[:], -1.0,
                                                           hvc(f"hb14{h}"),
                                                           op0=ALU.mult, op1=ALU.add)
                            ST["scl"][(4, s)][h] = sclT
                            ST["bis"][(4, s)][h] = bisT
                            ST["A4"][s][h] = A4T
                            ST["B4z"][s][h] = B4T
                        continue
                    # products for layer li+1
                    nli = li + 1
                    facT = coefp.tile([128, 1], f32, tag=f"fac{li}{k}")
                    e.tensor_tensor(facT[:], hvc(f"fac{nli}"), r_ap, op=ALU.mult)
                    lbase = {2: base2, 3: base3, 4: base4}[nli]
                    lcols = {2: 384, 3: 512, 4: 2048}[nli]
                    lw = wsc.tile([128, lcols], bf16, tag=f"l{nli}y")
                    nc.vector.tensor_scalar(lw[:], lbase[:], facT[:, 0:1], None,
                                            op0=ALU.mult)
                    ST[f"l{nli}y"][s] = lw
                    mT = {2: m2t, 3: m3t, 4: m4t}[nli]
                    rows = {2: 32, 3: 64, 4: 128}[nli]
                    nmm = 6 if nli == 4 else 3
                    for j in range(nmm):
                        nc.tensor.matmul(
                            accp[:, 4 + j:5 + j],
                            mT[0:rows, 128 * j:128 * j + 128],
                            mpb_ap[0:rows, 0:1],
                            start=True, stop=True)
                    negr = coefp.tile([128, 1], f32, tag=f"negr{li}{k}")
                    e.tensor_scalar(negr[:], r_ap, -1.0, None, op0=ALU.mult)
                    if nli < 4:
                        bvT = coefp.tile([128, 1], f32, tag=f"bv{nli}{k}")
                        nc.vector.scalar_tensor_tensor(bvT[:], accp[:, 4:5],
                                                       negr[:, 0:1],
                                                       hvc(f"W1n{nli}"),
                                                       op0=ALU.mult, op1=ALU.add)
                        e0T = coefp.tile([128, 1], f32, tag=f"e0{nli}{k}")
                        nc.vector.scalar_tensor_tensor(e0T[:], accp[:, 5:6],
                                                       negr[:, 0:1],
                                                       hvc(f"W1e0{nli}"),
                                                       op0=ALU.mult, op1=ALU.add)
                        eLT = coefp.tile([128, 1], f32, tag=f"eL{nli}{k}")
                        nc.vector.scalar_tensor_tensor(eLT[:], accp[:, 6:7],
                                                       negr[:, 0:1],
                                                       hvc(f"W1eL{nli}"),
                                                       op0=ALU.mult, op1=ALU.add)
                        ST[f"bvec{nli}"][s] = bvT[:]
                        ST[f"e0{nli}"][s] = e0T[:]
                        ST[f"eL{nli}"][s] = eLT[:]
                    else:
                        bvs, e0s, eLs = [], [], []
                        for h in range(2):
                            bvT = coefp.tile([128, 1], f32, tag=f"bv4{h}{k}")
                            nc.vector.scalar_tensor_tensor(
                                bvT[:], accp[:, 4 + 3 * h:5 + 3 * h],
                                negr[:, 0:1], hvc(f"W1n4{h}"),
                                op0=ALU.mult, op1=ALU.add)
                            e0T = coefp.tile([128, 1], f32, tag=f"e04{h}{k}")
                            nc.vector.scalar_tensor_tensor(
                                e0T[:], accp[:, 5 + 3 * h:6 + 3 * h],
                                negr[:, 0:1], hvc(f"W1e04{h}"),
                                op0=ALU.mult, op1=ALU.add)
                            eLT = coefp.tile([128, 1], f32, tag=f"eL4{h}{k}")
                            nc.vector.scalar_tensor_tensor(
                                eLT[:], accp[:, 6 + 3 * h:7 + 3 * h],
                                negr[:, 0:1], hvc(f"W1eL4{h}"),
                                op0=ALU.mult, op1=ALU.add)
                            bvs.append(bvT[:])
                            e0s.append(e0T[:])
                            eLs.append(eLT[:])
                        ST["bvec4"][s] = bvs
                        ST["e04"][s] = e0s
                        ST["eL4"][s] = eLs

            # --- main schedule (woven, cross-sample staggered) -----------
            def load_l1(s):
                ximt = xp.tile([128, QW], bf16, tag="xim")
                nc.sync.dma_start(ximt[:], xim_d[s])
                sy = slotp.tile([128, 16], f32, tag="sy1")
                sq_ = slotp.tile([128, 16], f32, tag="sq1")
                for _, fn in l1_conv_chunks(ximt, sy, sq_):
                    fn()
                return sy, sq_

            def chain12(s, sl):
                """ph2(L1, s) woven with L2 conv of s; returns L2 slots."""
                sy2 = slotp.tile([128, 16], f32, tag="sy2")
                sq2 = slotp.tile([128, 16], f32, tag="sq2")

                def post():
                    for p0 in (32, 64, 96):
                        nc.gpsimd.memset(c1[p0:p0 + 32, G1 + 16000:G1 + 16001], 0.0)
                pch = ph2_chunks(1, y1, G1, U1, c1, G1,
                                 ST["scl"][(1, s)][:], ST["bis"][(1, s)][:],
                                 post=post)
                cch = l2_conv_chunks(sl, ST["l2y"][s], ST["bvec2"][s],
                                     ST["e02"][s], ST["eL2"][s], sy2, sq2)
                weave(pch, cch)
                return sy2, sq2

            def chain23(s, sl, sy3, sq3):
                def post():
                    for p0 in (64, 96):
                        nc.gpsimd.memset(
                            c2[p0:p0 + 32, sl * W2B + G2 + 8000:sl * W2B + G2 + 8001], 0.0)
                pch = ph2_chunks(2, y2, sl * W2B + G2, V2, c2, sl * W2B + G2,
                                 ST["scl"][(2, s)][:], ST["bis"][(2, s)][:],
                                 post=post)
                cch = l3_conv_chunks(sl, ST["l3y"][s], ST["bvec3"][s],
                                     ST["e03"][s], ST["eL3"][s], sy3, sq3)
                weave(pch, cch)

            def chain34(s, sl, sy4, sq4):
                pch = ph2_chunks(3, y3, sl * W3B + G3, T3, c3, sl * W3B + G3,
                                 ST["scl"][(3, s)][:], ST["bis"][(3, s)][:])
                cch = l4_conv_chunks(sl, ST["l4y"][s], ST["bvec4"][s],
                                     ST["e04"][s], ST["eL4"][s], sy4, sq4)
                weave(pch, cch)

            def out4(s, sl):
                o4 = sl * 2002
                for h in range(2):
                    for _, fn in ph2_chunks(4, y4, o4 + 1001 * h, T4, c4,
                                            o4 + 1001 * h,
                                            ST["scl"][(4, s)][h][:],
                                            ST["bis"][(4, s)][h][:]):
                        fn()
                    nc.gpsimd.tensor_scalar(
                        y4[:, o4 + 1001 * h:o4 + 1001 * h + T4],
                        y4[:, o4 + 1001 * h:o4 + 1001 * h + T4],
                        ST["A4"][s][h][:, 0:1], ST["B4z"][s][h][:, 0:1],
                        op0=ALU.mult, op1=ALU.add)
                    nc.vector.scalar_tensor_tensor(
                        c4[:, o4 + 1001 * h:o4 + 1001 * h + T4],
                        c4[:, o4 + 1001 * h:o4 + 1001 * h + T4],
                        hvc(f"hC4{h}"),
                        y4[:, o4 + 1001 * h:o4 + 1001 * h + T4],
                        op0=ALU.mult, op1=ALU.add)
                    pstt = pstp.tile([128, 1024], bf16, tag="pst")
                    for j in range(8):
                        t0 = 128 * j
                        bw = min(128, T4 - t0)
                        nc.tensor.transpose(
                            pstt[0:bw, 128 * j:128 * j + 128],
                            c4[:, o4 + 1001 * h + t0:o4 + 1001 * h + t0 + bw],
                            eye[:])
                    ot = outp.tile([128, 1024], bf16, tag="outT")
                    nc.vector.tensor_copy(ot[:, 0:896], pstt[:, 0:896])
                    nc.vector.tensor_copy(ot[0:105, 896:1024],
                                          pstt[0:105, 896:1024])
                    src = ot[:, 0:896].rearrange("p (b c) -> p b c", c=128)
                    dst = out_d[s, 0:896, 128 * h:128 * h + 128]
                    dst = dst.rearrange("(b p) c -> p b c", p=128)
                    nc.sync.dma_start(dst, src)
                    nc.sync.dma_start(
                        out_d[s, 896:1001, 128 * h:128 * h + 128],
                        ot[0:105, 896:1024])

            NCH2 = (len(L2_CH) + 1, len(L2_CH))
            NCH3 = (len(L3_CH), len(L3_CH))
            slt = {}

            def seg_l1(s):
                sy, sq_ = load_l1(s)
                emit_coefs(1, [(s, sy, sq_, (len(L1_CH), len(L1_CH)))])

            def seg_12(s):
                slt[(2, s)] = chain12(s, s % 2)

            def seg_23(pss):
                emit_coefs(2, [(s, *slt[(2, s)], NCH2) for s in pss])
                for s in pss:
                    sy3 = slotp.tile([128, 16], f32, tag="sy3")
                    sq3 = slotp.tile([128, 16], f32, tag="sq3")
                    slt[(3, s)] = (sy3, sq3)
                    chain23(s, s % 2, sy3, sq3)

            def seg_34(pss):
                emit_coefs(3, [(s, *slt[(3, s)], NCH3) for s in pss])
                for s in pss:
                    sy4 = slotp.tile([128, 16], f32, tag="sy4")
                    sq4 = slotp.tile([128, 16], f32, tag="sq4")
                    slt[(4, s)] = (sy4, sq4)
                    chain34(s, s % 2, sy4, sq4)

            def seg_out(pss):
                emit_coefs(4, [(s, *slt[(4, s)], (2, 2)) for s in pss])
                for s in pss:
                    out4(s, s % 2)

            # staggered order: next pair's L1/L2 fills this pair's coef+ph2 gaps
            ORDER = os.environ.get("KN_ORD", "C")
            if ORDER == "A":
                seg_l1(0); seg_12(0); seg_l1(1); seg_12(1); seg_l1(2)
                seg_23([0, 1]); seg_12(2); seg_l1(3); seg_34([0, 1])
                seg_12(3); seg_out([0, 1]); seg_23([2, 3]); seg_34([2, 3])
                seg_out([2, 3])
            elif ORDER == "B":
                seg_l1(0); seg_12(0); seg_l1(1); seg_12(1); seg_l1(2)
                seg_23([0, 1]); seg_12(2); seg_34([0, 1]); seg_l1(3)
                seg_12(3); seg_out([0, 1]); seg_23([2, 3]); seg_34([2, 3])
                seg_out([2, 3])
            elif ORDER == "C":
                seg_l1(0); seg_12(0); seg_l1(1); seg_12(1); seg_l1(2)
                seg_23([0, 1]); seg_12(2); seg_l1(3); seg_34([0, 1])
                seg_12(3); seg_23([2, 3]); seg_out([0, 1]); seg_34([2, 3])
                seg_out([2, 3])

    split_multi_waits(nc)
    return nc


def kernel(**inputs):
    global LAST_RESULTS
    host, eps_eff, ix = _host_prep(inputs)
    nv = host["hv"].shape[1]

    key = tuple(round(e, 14) for e in eps_eff)
    if key not in _CACHE:
        _CACHE.clear()
        _CACHE[key] = _build_program(eps_eff, ix, nv)
    nc = _CACHE[key]

    x = np.asarray(inputs["x"], np.float32)
    in_maps = []
    for c in range(N_CORES):
        xs = x[c * BPC:(c + 1) * BPC]
        m = {"xim": _build_im2col(xs)}
        m.update(host)
        in_maps.append(m)

    trace = os.environ.get("KERNEL_TRACE", "0") == "1"
    kw = {}
    if trace:
        import importlib.util
        if importlib.util.find_spec("antenv") is not None and \
                importlib.util.find_spec("antenv.axon_hooks") is not None:
            kw = dict(trace=True, trace_cores=list(range(N_CORES)))
    res = run_bass_kernel_spmd(nc, in_maps, core_ids=list(range(N_CORES)), **kw)
    LAST_RESULTS = res
    out = np.concatenate(
        [res.results[c]["out"].astype(np.float32) for c in range(N_CORES)],
        axis=0)
    return out

